# revision 3
# baseline (speedup 1.0000x reference)
"""Multi-head attention (BaselineAttention) Bass kernel for 8 trn2 NeuronCores.

Problem: x[4,2048,1024], per-head Wq/Wk/Wv [16,1024,64] (+biases), Wo[1024,1024]+bo.
Sharding: core c -> batch b=c//2, head-group g=c%2 (8 heads each).
Each core computes y_partial[b] = sum_{h in group} softmax(qk^T/8) v @ Wo_rows(h).
Host combines: y[b] = part[2b] + part[2b+1] + bo + bv@Wo  (bv folded out of device).

Device algorithm per core (all matmul operands bf16; psum f32):
  x resident in SBUF [128, 8kt, 2048]; per pair (2 heads): qT/kT [128=(j,e), s]
  via W^T@x; v[t,(j,e)] via x^T@Wv with an appended ones column (vA [.,tt,j,65]).
  Attention per (head, s-chunk 512): scores^T[t,s] = kT^T qT (K=64); exp on ACT
  -> att bf16 [t, s]; o_aug[s, e|r] = att^T-stationary @ [v|1]-moving (N=65,
  psum-accumulated over 16 t-tiles); normalize with DVE per-partition scalar
  1/r; PE-transpose o_n -> onorm[(j,e), pair, s]; out-proj y = onorm^T @ Wo.
  Pair-pipelined schedule: QKV of pair p+1 and out-proj fill PE while ACT
  computes exp of pair p (exp is the co-bottleneck at ~267us vs PE ~285us).
"""
import numpy as np

B, S, DIM, H, DH = 4, 2048, 1024, 16, 64
NCORES = 8
HPC = H // 2          # heads per core = 8
NPAIR = HPC // 2      # head pairs per core = 4
NT = S // 128         # t-tiles = 16
NSQ = S // 512        # s-chunks of 512 = 4
NKT = DIM // 128      # d-tiles = 8
SCALE = 1.0 / float(np.sqrt(DH))

_CACHE = {}


def _build(repeat=1, debug_taps=False):
    from collections import deque
    import concourse.bass as bass  # noqa: F401
    import concourse.mybir as mybir
    import concourse.tile as tile
    from concourse import bacc

    f32 = mybir.dt.float32
    bf16 = mybir.dt.bfloat16
    AF = mybir.ActivationFunctionType

    nc = bacc.Bacc("TRN2", target_bir_lowering=False, debug=False,
                   num_devices=NCORES)

    xT_d = nc.dram_tensor("xT", [DIM, S], bf16, kind="ExternalInput")
    wq_d = nc.dram_tensor("wq", [NPAIR, DIM, 128], bf16, kind="ExternalInput")
    wk_d = nc.dram_tensor("wk", [NPAIR, DIM, 128], bf16, kind="ExternalInput")
    wv_d = nc.dram_tensor("wv", [NPAIR, DIM, 128], bf16, kind="ExternalInput")
    bq_d = nc.dram_tensor("bq", [128, NPAIR], f32, kind="ExternalInput")
    bk_d = nc.dram_tensor("bk", [128, NPAIR], f32, kind="ExternalInput")
    wo_d = nc.dram_tensor("wo", [128, NPAIR, DIM], bf16, kind="ExternalInput")
    eye_d = nc.dram_tensor("eye", [128, 128], bf16, kind="ExternalInput")
    y_d = nc.dram_tensor("y", [S, DIM], f32, kind="ExternalOutput")
    if debug_taps:
        dbg_qT = nc.dram_tensor("dbg_qT", [128, S], bf16, kind="ExternalOutput")
        dbg_kT = nc.dram_tensor("dbg_kT", [128, S], bf16, kind="ExternalOutput")
        dbg_vA = nc.dram_tensor("dbg_vA", [128, NT, 2, 65], bf16,
                                kind="ExternalOutput")
        dbg_att = nc.dram_tensor("dbg_att", [128, NT, 512], bf16,
                                 kind="ExternalOutput")
        dbg_on = nc.dram_tensor("dbg_on", [128, NPAIR, S], bf16,
                                kind="ExternalOutput")

    ctr = [0]

    def nm(pfx):
        ctr[0] += 1
        return f"{pfx}_{ctr[0]}"

    with tile.TileContext(nc) as tc:
        with tc.tile_pool(name="persist", bufs=1) as pp, \
             tc.tile_pool(name="work", bufs=1) as wp, \
             tc.tile_pool(name="ps", bufs=1, space="PSUM") as ps:
            # ---- persistent SBUF ----
            x_sb = pp.tile([128, NKT, S], bf16)
            eye = pp.tile([128, 128], bf16)
            bqs = pp.tile([128, NPAIR], f32)
            bks = pp.tile([128, NPAIR], f32)
            wo_sb = pp.tile([128, NPAIR, DIM], bf16)
            onorm = pp.tile([128, NPAIR, S], bf16)   # [(j,e), pair, s]
            nc.sync.dma_start(out=eye, in_=eye_d.ap())
            nc.sync.dma_start(out=bqs, in_=bq_d.ap())
            nc.sync.dma_start(out=bks, in_=bk_d.ap())
            nc.sync.dma_start(out=wo_sb, in_=wo_d.ap())

            for rep in range(repeat):
                xT_src = xT_d.ap().rearrange("(kt p) s -> p kt s", p=128)
                for kt in range(NKT):
                    nc.sync.dma_start(out=x_sb[:, kt, :], in_=xT_src[:, kt, :])

                wtiles = {}

                def w_dmas(p):
                    for pfx, wd in (("q", wq_d), ("k", wk_d), ("v", wv_d)):
                        w = wp.tile([128, NKT, 128], bf16, tag=f"w{pfx}",
                                    bufs=2, name=nm(f"w{pfx}{p}"))
                        nc.sync.dma_start(
                            out=w,
                            in_=wd.ap()[p].rearrange("(kt pp) m -> pp kt m",
                                                     pp=128))
                        wtiles[(pfx, p)] = w

                qkts = {}

                def qkv_pieces(p):
                    """12 closures: k-proj x4, q-proj x4, v-proj x4."""
                    qT = wp.tile([128, S], bf16, tag="qT", bufs=2,
                                 name=nm(f"qT{p}"))
                    kT = wp.tile([128, S], bf16, tag="kT", bufs=2,
                                 name=nm(f"kT{p}"))
                    vA = wp.tile([128, NT, 2, 65], bf16, tag="vA", bufs=2,
                                 name=nm(f"vA{p}"))
                    qkts[p] = (qT, kT, vA)
                    pieces = []

                    def ones_piece():
                        nc.gpsimd.memset(vA[:, :, :, 64:65], 1.0)

                    def qk_piece(pfx, dst, bias, sq):
                        def go():
                            qk_ps = ps.tile([128, 512], f32, tag="qk", bufs=1,
                                            name=nm(f"ps{pfx}{p}{sq}"))
                            w = wtiles[(pfx, p)]
                            for kt in range(NKT):
                                nc.tensor.matmul(
                                    qk_ps, w[:, kt, :],
                                    x_sb[:, kt, sq * 512:(sq + 1) * 512],
                                    start=(kt == 0), stop=(kt == NKT - 1))
                            nc.vector.tensor_scalar_add(
                                out=dst[:, sq * 512:(sq + 1) * 512],
                                in0=qk_ps, scalar1=bias[:, p:p + 1])
                        return go

                    def v_piece(ttg):
                        def go():
                            v_ps = ps.tile([128, 512], f32, tag="vps", bufs=1,
                                           name=nm(f"psv{p}{ttg}"))
                            v_ps4 = v_ps.rearrange("p (t m) -> p t m", t=4)
                            w = wtiles[("v", p)]
                            for t4 in range(4):
                                tt = ttg * 4 + t4
                                for kt in range(NKT):
                                    nc.tensor.matmul(
                                        v_ps4[:, t4, :],
                                        x_sb[:, kt, tt * 128:(tt + 1) * 128],
                                        w[:, kt, :],
                                        start=(kt == 0), stop=(kt == NKT - 1))
                            nc.vector.tensor_copy(
                                vA[:, ttg * 4:(ttg + 1) * 4, :, 0:64],
                                v_ps.rearrange("p (t j e) -> p t j e",
                                               t=4, j=2))
                        return go

                    pieces.append(ones_piece)
                    for sq in range(NSQ):
                        pieces.append(qk_piece("k", kT, bks, sq))
                    for sq in range(NSQ):
                        pieces.append(qk_piece("q", qT, bqs, sq))
                    for ttg in range(4):
                        pieces.append(v_piece(ttg))
                    return pieces

                # deferred-work queues: hot = attention back-half chain
                # (must run ASAP), cold = PE filler (qkv / out-proj).
                hot = deque()
                cold = deque()

                def drain_hot():
                    while hot:
                        hot.popleft()()

                def pop_cold(k=1):
                    for _ in range(k):
                        if cold:
                            cold.popleft()()

                def oproj_sq(sq):
                    """8 closures, one per (st, nh) column block of y."""
                    out = []
                    for sti in range(4):
                        st = sq * 4 + sti
                        for nh in range(2):
                            def go(st=st, nh=nh):
                                tag = "qk" if nh == 0 else "vps"
                                y_ps = ps.tile([128, 512], f32, tag=tag,
                                               bufs=1, name=nm(f"psy{st}{nh}"))
                                for p in range(NPAIR):
                                    nc.tensor.matmul(
                                        y_ps,
                                        onorm[:, p, st * 128:(st + 1) * 128],
                                        wo_sb[:, p, nh * 512:(nh + 1) * 512],
                                        start=(p == 0), stop=(p == NPAIR - 1))
                                y_sb = wp.tile([128, 512], f32, tag="ysb",
                                               bufs=2, name=nm(f"ysb{st}{nh}"))
                                nc.vector.tensor_copy(y_sb, y_ps)
                                nc.sync.dma_start(
                                    out=y_d.ap()[st * 128:(st + 1) * 128,
                                                 nh * 512:(nh + 1) * 512],
                                    in_=y_sb)
                            out.append(go)
                    return out

                def attn_back(p, j, sq, att_t, last_slot):
                    """attn@v + normalize + transpose (+ copy & oproj when
                    this closes a (pair, sq) group)."""
                    def go():
                        qT, kT, vA = qkts[p]
                        o_ps = ps.tile([128, 4, 65], f32, tag="ops", bufs=1,
                                       name=nm(f"o{p}{j}{sq}"))
                        for sti in range(4):
                            for tt in range(NT):
                                nc.tensor.matmul(
                                    o_ps[:, sti, :],
                                    att_t[:, tt, sti * 128:(sti + 1) * 128],
                                    vA[:, tt, j, :],
                                    start=(tt == 0), stop=(tt == NT - 1))
                        rinv = wp.tile([128, 4], f32, tag="rinv", bufs=2,
                                       name=nm(f"ri{p}{j}{sq}"))
                        nc.vector.reciprocal(rinv, o_ps[:, :, 64])
                        o_n = wp.tile([128, 4, 64], bf16, tag="on", bufs=2,
                                      name=nm(f"on{p}{j}{sq}"))
                        for sti in range(4):
                            nc.vector.tensor_scalar_mul(
                                out=o_n[:, sti, :],
                                in0=o_ps[:, sti, 0:64],
                                scalar1=rinv[:, sti:sti + 1])
                        pt = pt_for[(p, sq)]
                        for sti in range(4):
                            nc.tensor.matmul(
                                pt[64 * j:64 * (j + 1), sti, :],
                                o_n[:, sti, :], eye, is_transpose=True)
                        if j == 1:
                            nc.vector.tensor_copy(
                                onorm[:, p, sq * 512:(sq + 1) * 512],
                                pt.rearrange("p st s -> p (st s)"))
                            if last_slot:
                                cold.extend(oproj_sq(sq))
                    return go

                pt_for = {}

                w_dmas(0)
                for slot in range(NPAIR + 1):
                    if slot < NPAIR - 1:
                        w_dmas(slot + 1)
                    if slot < NPAIR:
                        cold.extend(qkv_pieces(slot))
                    if slot == 0:
                        pop_cold(13)
                        continue
                    p = slot - 1
                    last_slot = slot == NPAIR
                    for sq in range(NSQ):
                        pt_for[(p, sq)] = ps.tile(
                            [128, 4, 128], bf16, tag="pt", bufs=1,
                            name=nm(f"pt{p}{sq}"))
                        for j in range(2):
                            qT, kT, vA = qkts[p]
                            att_t = wp.tile([128, NT, 512], bf16, tag="att",
                                            bufs=3, name=nm(f"att{p}{j}{sq}"))
                            lo = 64 * j
                            for g in range(NT // 2):
                                sc = ps.tile([128, 2, 512], f32, tag="sc",
                                             bufs=2, name=nm(f"sc{p}{j}{sq}{g}"))
                                for i in range(2):
                                    tt = 2 * g + i
                                    nc.tensor.matmul(
                                        sc[:, i, :],
                                        kT[lo:lo + 64, tt * 128:(tt + 1) * 128],
                                        qT[lo:lo + 64, sq * 512:(sq + 1) * 512],
                                        start=True, stop=True)
                                nc.scalar.activation(
                                    att_t[:, 2 * g:2 * g + 2, :], sc,
                                    AF.Exp, scale=SCALE)
                                if g == 1:
                                    drain_hot()
                                elif g in (3, 5, 7):
                                    pop_cold()
                            hot.append(attn_back(p, j, sq, att_t, last_slot))
                            if debug_taps and p == 0 and j == 0 and sq == 0:
                                def dbg0(att_t=att_t):
                                    qT, kT, vA = qkts[0]
                                    nc.sync.dma_start(out=dbg_qT.ap(), in_=qT)
                                    nc.sync.dma_start(out=dbg_kT.ap(), in_=kT)
                                    nc.sync.dma_start(out=dbg_vA.ap(), in_=vA)
                                    nc.sync.dma_start(out=dbg_att.ap(),
                                                      in_=att_t)
                                hot.append(dbg0)
                drain_hot()
                while cold:
                    pop_cold()
                if debug_taps:
                    nc.sync.dma_start(out=dbg_on.ap(), in_=onorm)
    nc.compile()
    return nc


def _get_nc():
    if "nc" not in _CACHE:
        _CACHE["nc"] = _build()
    return _CACHE["nc"]


def _bf16(a):
    import ml_dtypes
    return np.ascontiguousarray(a).astype(ml_dtypes.bfloat16)


def make_in_maps(x, Wq, Wk, Wv, bq, bk, bv, Wo, bo):
    eye = np.eye(128, dtype=np.float32)
    in_maps = []
    for c in range(NCORES):
        b, g = c // 2, c % 2
        hs = slice(g * HPC, (g + 1) * HPC)
        # weights pair-packed: [pair, DIM, 128=(j,e)]
        wq3 = Wq[hs].reshape(NPAIR, 2, DIM, DH).transpose(0, 2, 1, 3) \
            .reshape(NPAIR, DIM, 128)
        wk3 = Wk[hs].reshape(NPAIR, 2, DIM, DH).transpose(0, 2, 1, 3) \
            .reshape(NPAIR, DIM, 128)
        wv3 = Wv[hs].reshape(NPAIR, 2, DIM, DH).transpose(0, 2, 1, 3) \
            .reshape(NPAIR, DIM, 128)
        # wo: [128=(j,e), pair, DIM]
        wo3 = Wo[g * 512:(g + 1) * 512, :].reshape(NPAIR, 128, DIM) \
            .transpose(1, 0, 2)
        in_maps.append({
            "xT": _bf16(x[b].T),
            "wq": _bf16(wq3),
            "wk": _bf16(wk3),
            "wv": _bf16(wv3),
            "bq": np.ascontiguousarray(bq[hs].reshape(NPAIR, 128).T),
            "bk": np.ascontiguousarray(bk[hs].reshape(NPAIR, 128).T),
            "wo": _bf16(wo3),
            "eye": _bf16(eye),
        })
    return in_maps


def combine(results, bv, Wo, bo):
    const = bv.reshape(DIM) @ Wo + bo          # [DIM]
    y = np.empty((B, S, DIM), dtype=np.float32)
    for b in range(B):
        y[b] = results[2 * b]["y"] + results[2 * b + 1]["y"] + const
    return y


def kernel(x, Wq, Wk, Wv, bq, bk, bv, Wo, bo):
    import time
    from concourse.bass_utils import run_bass_kernel_spmd
    x, Wq, Wk, Wv, bq, bk, bv, Wo, bo = [
        np.asarray(a, dtype=np.float32)
        for a in (x, Wq, Wk, Wv, bq, bk, bv, Wo, bo)]
    nc = _get_nc()
    in_maps = make_in_maps(x, Wq, Wk, Wv, bq, bk, bv, Wo, bo)
    last = None
    for attempt in range(3):
        try:
            res = run_bass_kernel_spmd(nc, in_maps,
                                       core_ids=list(range(NCORES)))
            return combine(res.results, bv, Wo, bo)
        except Exception as e:  # transient NRT_EXEC_UNIT_UNRECOVERABLE wedges
            last = e
            time.sleep(75)
    raise last
